# revision 3
# baseline (speedup 1.0000x reference)
"""Trainium2 Bass kernel for DiseaseKnowledgeModule.

Math (per token z in R^d, d=1024, 14 diseases x 2 states):
  score = z @ flat_memory.T / sqrt(d)                 [28]
  p     = softmax over states [:,1]  == sigmoid((s1 - s0))        [14]
  mlc   = max over patches of p                       [B, 14]
  R     = p @ M_present                               [d]
  gate  = sigmoid([z, R] @ gate_w.T + gate_b)         [d]
  z_out = z + gate * R

Device formulation (feature-major / transposed activations):
  sd    = Wdiff^T-contraction: (m1 - m0) . z          -> one matmul, K=d
  p     = sigmoid(sd / 32)
  gate^T= sigmoid(Wg1 @ z^T + (Wg2 @ Mp^T) @ p^T + b) -> K=d + K=14 (folded)
  R^T   = Mp^T-par matmul: K=14
  zout^T= z^T + gate^T * R^T

Sharding: data-parallel over batch B=32 across 8 cores (4 batches/core).
All matmuls in fp32r (full PE speed, ~1e-4 relative accuracy here).
"""
import sys
import numpy as np

sys.path.insert(0, "/opt/trn_rl_repo")

import concourse.bass as bass
import concourse.mybir as mybir
from concourse.bass_utils import run_bass_kernel_spmd

F32 = mybir.dt.float32
F32R = mybir.dt.float32r
AX = mybir.AxisListType
ALU = mybir.AluOpType
ACTF = mybir.ActivationFunctionType

B, S, D = 32, 1024, 1024
ND, NS = 14, 2
CORES = 8
B_LOC = B // CORES            # 4 batches per core
T = B_LOC * S                 # 4096 tokens per core
NTILE = 512                   # tokens per T-tile (one PSUM bank of fp32)
NT = T // NTILE               # 8 T-tiles
KC = D // 128                 # 8 contraction chunks of 128
JC = D // 128                 # 8 output d-tiles of 128
XBUF = 3                      # xt triple-buffer
SCALE = 1.0 / np.sqrt(np.float32(D))

_prog_cache = {}


def _op_numbering():
    """Per-engine op indices (1-based cumulative semaphore values)."""
    pe, act, dve = {}, {}, {}
    n = 0
    n += 1; pe[("score", 0)] = n
    for t in range(NT):
        if t + 1 < NT:
            n += 1; pe[("score", t + 1)] = n
        for j in range(JC):
            n += 1; pe[("gate", t, j)] = n
            n += 1; pe[("r", t, j)] = n
    n = 0
    for t in range(NT):
        n += 1; act[("psig", t)] = n
        for j in range(JC):
            n += 1; act[("gsig", t, j)] = n
    n = 0
    for t in range(NT):
        n += 1; dve[("rmax", t)] = n
        for j in range(JC):
            n += 1; dve[("mul", t, j)] = n
            n += 1; dve[("add", t, j)] = n
    return pe, act, dve


def _build_program():
    pe_i, act_i, dve_i = _op_numbering()
    nc = bass.Bass()

    xT = nc.dram_tensor("xT", [D, T], F32, kind="ExternalInput")
    wg1T = nc.dram_tensor("wg1T", [D, D], F32, kind="ExternalInput")
    wdT = nc.dram_tensor("wdT", [D, ND], F32, kind="ExternalInput")
    wg2pT = nc.dram_tensor("wg2pT", [ND, D], F32, kind="ExternalInput")
    mpres = nc.dram_tensor("mpres", [ND, D], F32, kind="ExternalInput")
    biasd = nc.dram_tensor("biasd", [128, JC], F32, kind="ExternalInput")
    zoT = nc.dram_tensor("zoT", [D, T], F32, kind="ExternalOutput")
    mlc8 = nc.dram_tensor("mlc8", [ND, NT], F32, kind="ExternalOutput")

    with nc.sbuf_tensor("xt", [128, XBUF * KC * NTILE], F32) as xt, \
         nc.sbuf_tensor("wg1", [128, KC * D], F32) as wg1, \
         nc.sbuf_tensor("wdiff", [128, KC * ND], F32) as wdiff, \
         nc.sbuf_tensor("wg2p", [ND, D], F32) as wg2p, \
         nc.sbuf_tensor("mpr", [ND, D], F32) as mpr, \
         nc.sbuf_tensor("bias_sb", [128, JC], F32) as bias_sb, \
         nc.sbuf_tensor("p_sb", [ND, 2 * NTILE], F32) as p_sb, \
         nc.sbuf_tensor("g_sb", [128, 2 * NTILE], F32) as g_sb, \
         nc.sbuf_tensor("zo", [128, 2 * JC * NTILE], F32) as zo, \
         nc.sbuf_tensor("mlc_sb", [ND, NT], F32) as mlc_sb, \
         nc.psum_tensor("sd_ps", [ND, 2 * NTILE], F32) as sd_ps, \
         nc.psum_tensor("gate_ps", [128, 2 * NTILE], F32) as gate_ps, \
         nc.psum_tensor("r_ps", [128, 2 * NTILE], F32) as r_ps, \
         nc.semaphore("wsem") as wsem, \
         nc.semaphore("xsem") as xsem, \
         nc.semaphore("osem") as osem, \
         nc.semaphore("pe_sem") as pe_sem, \
         nc.semaphore("act_sem") as act_sem, \
         nc.semaphore("dve_sem") as dve_sem, \
         nc.Block() as block:

        # ---- slice helpers ----
        def xt_sl(t, k):        # rhs [128, NTILE] for k-chunk of tile t
            o = (t % XBUF) * KC * NTILE + k * NTILE
            return xt.ap()[:, o:o + NTILE]

        def xt_tile(t):         # [128, KC, NTILE] destination for DMA
            o = (t % XBUF) * KC * NTILE
            return xt.ap()[:, o:o + KC * NTILE].rearrange(
                "p (k n) -> p k n", k=KC)

        def wg1_sl(k, j):
            o = k * D + j * 128
            return wg1.ap()[:, o:o + 128]

        def wd_sl(k):
            o = k * ND
            return wdiff.ap()[:, o:o + ND]

        def p_sl(t):
            o = (t % 2) * NTILE
            return p_sb.ap()[:, o:o + NTILE]

        def g_sl(j):
            o = (j % 2) * NTILE
            return g_sb.ap()[:, o:o + NTILE]

        def zo_sl(t, j):
            o = (t % 2) * JC * NTILE + j * NTILE
            return zo.ap()[:, o:o + NTILE]

        def zo_tile(t):
            o = (t % 2) * JC * NTILE
            return zo.ap()[:, o:o + JC * NTILE].rearrange(
                "p (j n) -> p j n", j=JC)

        def sd_sl(t):
            o = (t % 2) * NTILE
            return sd_ps.ap()[:, o:o + NTILE]

        def gate_sl(j):
            o = (j % 2) * NTILE
            return gate_ps.ap()[:, o:o + NTILE]

        def r_sl(j):
            o = (j % 2) * NTILE
            return r_ps.ap()[:, o:o + NTILE]

        # ---- SP: all DMA ----
        @block.sync
        def _(sync):
            sync.dma_start(
                wg1.ap().rearrange("p (k m) -> p k m", k=KC).bitcast(F32R),
                wg1T.rearrange("(k p) m -> p k m", p=128).bitcast(F32R),
            ).then_inc(wsem, 16)
            sync.dma_start(
                wdiff.ap().rearrange("p (k m) -> p k m", k=KC).bitcast(F32R),
                wdT.rearrange("(k p) m -> p k m", p=128).bitcast(F32R),
            ).then_inc(wsem, 16)
            sync.dma_start(wg2p.ap().bitcast(F32R),
                           wg2pT[:, :].bitcast(F32R)).then_inc(wsem, 16)
            sync.dma_start(mpr.ap().bitcast(F32R),
                           mpres[:, :].bitcast(F32R)).then_inc(wsem, 16)
            sync.dma_start(bias_sb.ap(), biasd[:, :]).then_inc(wsem, 16)

            xsrc = xT.rearrange("(k p) (t n) -> p k t n", p=128, n=NTILE)
            for t in range(min(XBUF, NT)):
                sync.dma_start(xt_tile(t).bitcast(F32R),
                               xsrc[:, :, t, :].bitcast(F32R)).then_inc(xsem, 16)
            zdst = zoT.rearrange("(j p) (t n) -> p j t n", p=128, n=NTILE)
            for t in range(NT):
                sync.wait_ge(dve_sem, dve_i[("add", t, JC - 1)])
                sync.dma_start(zdst[:, :, t, :], zo_tile(t)).then_inc(osem, 16)
                if t + XBUF < NT:
                    sync.dma_start(xt_tile(t + XBUF).bitcast(F32R),
                                   xsrc[:, :, t + XBUF, :].bitcast(F32R)
                                   ).then_inc(xsem, 16)
            sync.wait_ge(dve_sem, dve_i[("rmax", NT - 1)])
            sync.dma_start(mlc8[:, :], mlc_sb.ap()).then_inc(osem, 16)
            sync.wait_ge(osem, 16 * (NT + 1))

        # ---- PE: all matmuls (fp32r) ----
        @block.tensor
        def _(tensor):
            def score(t):
                tensor.wait_ge(xsem, 16 * (t + 1))
                if t >= 2:
                    tensor.wait_ge(act_sem, act_i[("psig", t - 2)])
                for k in range(KC):
                    mm = nc.tensor.matmul(
                        sd_sl(t), wd_sl(k).bitcast(F32R),
                        xt_sl(t, k).bitcast(F32R),
                        start=(k == 0), stop=(k == KC - 1))
                mm.then_inc(pe_sem, 1)

            tensor.wait_ge(wsem, 80)
            score(0)
            for t in range(NT):
                if t + 1 < NT:
                    score(t + 1)
                tensor.wait_ge(act_sem, act_i[("psig", t)])
                for j in range(JC):
                    # gate_pre_j = sum_k Wg1[k,j]^T x[k] + Wg2p[j]^T p
                    if j >= 2:
                        tensor.wait_ge(act_sem, act_i[("gsig", t, j - 2)])
                    elif t >= 1:
                        tensor.wait_ge(act_sem, act_i[("gsig", t - 1, j + JC - 2)])
                    for k in range(KC):
                        nc.tensor.matmul(
                            gate_sl(j), wg1_sl(k, j).bitcast(F32R),
                            xt_sl(t, k).bitcast(F32R),
                            start=(k == 0), stop=False)
                    mm = nc.tensor.matmul(
                        gate_sl(j),
                        wg2p.ap()[:, j * 128:(j + 1) * 128].bitcast(F32R),
                        p_sl(t).bitcast(F32R),
                        start=False, stop=True)
                    mm.then_inc(pe_sem, 1)
                    # R_j = Mp[j]^T p
                    if j >= 2:
                        tensor.wait_ge(dve_sem, dve_i[("mul", t, j - 2)])
                    elif t >= 1:
                        tensor.wait_ge(dve_sem, dve_i[("mul", t - 1, j + JC - 2)])
                    mm = nc.tensor.matmul(
                        r_sl(j),
                        mpr.ap()[:, j * 128:(j + 1) * 128].bitcast(F32R),
                        p_sl(t).bitcast(F32R),
                        start=True, stop=True)
                    mm.then_inc(pe_sem, 1)

        # ---- ACT: sigmoids ----
        @block.scalar
        def _(scalar):
            for t in range(NT):
                scalar.wait_ge(pe_sem, pe_i[("score", t)])
                if t >= 2:
                    scalar.wait_ge(pe_sem, pe_i[("r", t - 2, JC - 1)])
                    scalar.wait_ge(dve_sem, dve_i[("rmax", t - 2)])
                nc.scalar.activation(
                    p_sl(t).bitcast(F32R), sd_sl(t), ACTF.Sigmoid,
                    scale=float(SCALE)).then_inc(act_sem, 1)
                for j in range(JC):
                    scalar.wait_ge(pe_sem, pe_i[("gate", t, j)])
                    if j >= 2:
                        scalar.wait_ge(dve_sem, dve_i[("mul", t, j - 2)])
                    elif t >= 1:
                        scalar.wait_ge(dve_sem, dve_i[("mul", t - 1, j + JC - 2)])
                    nc.scalar.activation(
                        g_sl(j), gate_sl(j), ACTF.Sigmoid,
                        bias=bias_sb.ap()[:, j:j + 1]).then_inc(act_sem, 1)

        # ---- DVE: max-reduce + elementwise epilogue ----
        @block.vector
        def _(vector):
            for t in range(NT):
                vector.wait_ge(act_sem, act_i[("psig", t)])
                nc.vector.tensor_reduce(
                    mlc_sb.ap()[:, t:t + 1], p_sl(t), axis=AX.X,
                    op=ALU.max).then_inc(dve_sem, 1)
                if t >= 2:
                    vector.wait_ge(osem, 16 * (t - 1))
                for j in range(JC):
                    vector.wait_ge(act_sem, act_i[("gsig", t, j)])
                    vector.wait_ge(pe_sem, pe_i[("r", t, j)])
                    nc.vector.tensor_mul(
                        zo_sl(t, j), g_sl(j), r_sl(j)).then_inc(dve_sem, 1)
                    nc.vector.tensor_add(
                        zo_sl(t, j), zo_sl(t, j),
                        xt_sl(t, j)).then_inc(dve_sem, 1)

    return nc


def _get_program():
    if "nc" not in _prog_cache:
        _prog_cache["nc"] = _build_program()
    return _prog_cache["nc"]


def kernel(z_fused, disease_knowledge, gate_w, gate_b):
    z_fused = np.asarray(z_fused, dtype=np.float32)
    dk = np.asarray(disease_knowledge, dtype=np.float32)
    gate_w = np.asarray(gate_w, dtype=np.float32)
    gate_b = np.asarray(gate_b, dtype=np.float32)

    m0 = dk[:, 0, :]                       # [14, D]
    m1 = dk[:, 1, :]                       # [14, D]
    wdT = np.ascontiguousarray((m1 - m0).T)            # [D, 14]
    wg1T = np.ascontiguousarray(gate_w[:, :D].T)       # [D, D]
    wg2pT = np.ascontiguousarray(m1 @ gate_w[:, D:].T)  # [14, D]
    mpres = np.ascontiguousarray(m1)                   # [14, D]
    biasd = np.ascontiguousarray(gate_b.reshape(JC, 128).T)  # [128, JC]

    nc = _get_program()
    in_maps = []
    for c in range(CORES):
        xs = np.ascontiguousarray(
            z_fused[c * B_LOC:(c + 1) * B_LOC].reshape(T, D).T)  # [D, T]
        in_maps.append({
            "xT": xs, "wg1T": wg1T, "wdT": wdT, "wg2pT": wg2pT,
            "mpres": mpres, "biasd": biasd,
        })

    res = run_bass_kernel_spmd(nc, in_maps, list(range(CORES)))

    z_out = np.empty((B, S, D), dtype=np.float32)
    mlc = np.empty((B, ND), dtype=np.float32)
    for c in range(CORES):
        r = res.results[c]
        z_out[c * B_LOC:(c + 1) * B_LOC] = np.ascontiguousarray(
            r["zoT"].T).reshape(B_LOC, S, D)
        m8 = r["mlc8"]                      # [14, NT] max per 512-token tile
        per_b = m8.reshape(ND, B_LOC, NT // B_LOC).max(axis=2)  # [14, 4]
        mlc[c * B_LOC:(c + 1) * B_LOC] = per_b.T
    return z_out, mlc


# revision 6
# speedup vs baseline: 1.1558x; 1.1558x over previous
"""Trainium2 Bass kernel for DiseaseKnowledgeModule.

Math (per token z in R^d, d=1024, 14 diseases x 2 states):
  score = z @ flat_memory.T / sqrt(d)                    [28]
  p     = softmax over states [:, 1] == sigmoid(s1 - s0) [14]
  mlc   = max over patches of p                          [B, 14]
  R     = p @ M_present                                  [d]
  gate  = sigmoid([z, R] @ gate_w.T + gate_b)            [d]
  z_out = z + gate * R

Device formulation (feature-major / transposed activations):
  sd    = Wdiff contraction: (m1 - m0) . z               one matmul, K=d
  p     = sigmoid(sd / sqrt(d))
  gate^T= sigmoid(Wg1 @ z^T + (M1 @ Wg2^T)^T-fold @ p^T + b)   K=d + K=14
  R^T   = M1-contraction: K=14
  zout^T= z^T + gate^T * R^T

Sharding: data-parallel over batch B=32 across 8 cores (4 batches/core).
Matmuls in bf16 (fp32 PSUM accumulate); residual add in fp32.
"""
import sys
import numpy as np

sys.path.insert(0, "/opt/trn_rl_repo")

import ml_dtypes
import concourse.bass as bass
import concourse.mybir as mybir
from concourse.bass_utils import run_bass_kernel_spmd

F32 = mybir.dt.float32
BF16 = mybir.dt.bfloat16
AX = mybir.AxisListType
ALU = mybir.AluOpType
ACTF = mybir.ActivationFunctionType
NP_BF16 = ml_dtypes.bfloat16

B, S, D = 32, 1024, 1024
ND, NS = 14, 2
CORES = 8
B_LOC = B // CORES            # 4 batches per core
T = B_LOC * S                 # 4096 tokens per core
NTILE = 512                   # tokens per T-tile (one PSUM bank of fp32)
NT = T // NTILE               # 8 T-tiles
KC = D // 128                 # 8 contraction chunks of 128
JC = D // 128                 # 8 output d-tiles of 128
XBUF = 3                      # bf16 x tile buffers
SCALE = 1.0 / float(np.sqrt(np.float32(D)))

_prog_cache = {}


def _op_numbering():
    """Per-engine op indices (1-based cumulative semaphore values)."""
    pe, act, dve = {}, {}, {}
    n = 0
    n += 1; pe[("score", 0)] = n
    for t in range(NT):
        if t + 1 < NT:
            n += 1; pe[("score", t + 1)] = n
        for j in range(JC):
            n += 1; pe[("gate", t, j)] = n
            n += 1; pe[("r", t, j)] = n
    n = 0
    for t in range(NT):
        n += 1; act[("psig", t)] = n
        for j in range(JC):
            n += 1; act[("gsig", t, j)] = n
    n = 0
    for t in range(NT):
        n += 1; dve[("rmax", t)] = n
        for j in range(JC):
            n += 1; dve[("mul", t, j)] = n
            n += 1; dve[("add", t, j)] = n
    return pe, act, dve


def _build_program():
    pe_i, act_i, dve_i = _op_numbering()
    nc = bass.Bass()

    xb16 = nc.dram_tensor("xb16", [D, T], BF16, kind="ExternalInput")
    xf32 = nc.dram_tensor("xf32", [D, T], F32, kind="ExternalInput")
    wg1T = nc.dram_tensor("wg1T", [D, D], BF16, kind="ExternalInput")
    wdT = nc.dram_tensor("wdT", [D, ND], BF16, kind="ExternalInput")
    wg2pT = nc.dram_tensor("wg2pT", [ND, D], BF16, kind="ExternalInput")
    mpres = nc.dram_tensor("mpres", [ND, D], BF16, kind="ExternalInput")
    biasd = nc.dram_tensor("biasd", [128, JC], F32, kind="ExternalInput")
    zoT = nc.dram_tensor("zoT", [D, T], F32, kind="ExternalOutput")
    mlc8 = nc.dram_tensor("mlc8", [ND, NT], F32, kind="ExternalOutput")

    from contextlib import ExitStack
    with ExitStack() as ctx:
        xb = ctx.enter_context(nc.sbuf_tensor("xb", [128, XBUF * KC * NTILE], BF16))
        xf = ctx.enter_context(nc.sbuf_tensor("xf", [128, 2 * KC * NTILE], F32))
        wg1 = ctx.enter_context(nc.sbuf_tensor("wg1", [128, KC * D], BF16))
        wdiff = ctx.enter_context(nc.sbuf_tensor("wdiff", [128, KC * ND], BF16))
        wg2p = ctx.enter_context(nc.sbuf_tensor("wg2p", [ND, D], BF16))
        mpr = ctx.enter_context(nc.sbuf_tensor("mpr", [ND, D], BF16))
        bias_sb = ctx.enter_context(nc.sbuf_tensor("bias_sb", [128, JC], F32))
        p_sb = ctx.enter_context(nc.sbuf_tensor("p_sb", [ND, 2 * NTILE], BF16))
        g_sb = ctx.enter_context(nc.sbuf_tensor("g_sb", [128, 2 * NTILE], F32))
        zo = ctx.enter_context(nc.sbuf_tensor("zo", [128, 2 * JC * NTILE], F32))
        mlc_sb = ctx.enter_context(nc.sbuf_tensor("mlc_sb", [ND, NT], F32))
        sd_ps = ctx.enter_context(nc.psum_tensor("sd_ps", [ND, 2 * NTILE], F32))
        gate_ps = ctx.enter_context(nc.psum_tensor("gate_ps", [128, 2 * NTILE], F32))
        r_ps = ctx.enter_context(nc.psum_tensor("r_ps", [128, 2 * NTILE], F32))
        wdsem = ctx.enter_context(nc.semaphore("wdsem"))
        wsem = ctx.enter_context(nc.semaphore("wsem"))
        xbsem = ctx.enter_context(nc.semaphore("xbsem"))
        xfsem = ctx.enter_context(nc.semaphore("xfsem"))
        osem = ctx.enter_context(nc.semaphore("osem"))
        pe_sem = ctx.enter_context(nc.semaphore("pe_sem"))
        act_sem = ctx.enter_context(nc.semaphore("act_sem"))
        dve_sem = ctx.enter_context(nc.semaphore("dve_sem"))
        block = ctx.enter_context(nc.Block(no_gpsimd_drain=True))

        # ---- slice helpers ----
        def xb_sl(t, k):        # bf16 rhs [128, NTILE] for k-chunk of tile t
            o = (t % XBUF) * KC * NTILE + k * NTILE
            return xb.ap()[:, o:o + NTILE]

        def xb_tile(t):
            o = (t % XBUF) * KC * NTILE
            return xb.ap()[:, o:o + KC * NTILE].rearrange(
                "p (k n) -> p k n", k=KC)

        def xf_sl(t, j):        # f32 residual [128, NTILE] for d-tile j
            o = (t % 2) * KC * NTILE + j * NTILE
            return xf.ap()[:, o:o + NTILE]

        def xf_tile(t):
            o = (t % 2) * KC * NTILE
            return xf.ap()[:, o:o + KC * NTILE].rearrange(
                "p (k n) -> p k n", k=KC)

        def wg1_sl(k, j):
            o = k * D + j * 128
            return wg1.ap()[:, o:o + 128]

        def wd_sl(k):
            o = k * ND
            return wdiff.ap()[:, o:o + ND]

        def p_sl(t):
            o = (t % 2) * NTILE
            return p_sb.ap()[:, o:o + NTILE]

        def g_sl(j):
            o = (j % 2) * NTILE
            return g_sb.ap()[:, o:o + NTILE]

        def zo_sl(t, j):
            o = (t % 2) * JC * NTILE + j * NTILE
            return zo.ap()[:, o:o + NTILE]

        def zo_tile(t):
            o = (t % 2) * JC * NTILE
            return zo.ap()[:, o:o + JC * NTILE].rearrange(
                "p (j n) -> p j n", j=JC)

        def sd_sl(t):
            o = (t % 2) * NTILE
            return sd_ps.ap()[:, o:o + NTILE]

        def gate_sl(j):
            o = (j % 2) * NTILE
            return gate_ps.ap()[:, o:o + NTILE]

        def r_sl(j):
            o = (j % 2) * NTILE
            return r_ps.ap()[:, o:o + NTILE]

        # ---- SP: all DMA ----
        @block.sync
        def _(sync):
            xbsrc = xb16.rearrange("(k p) (t n) -> p k t n", p=128, n=NTILE)
            xfsrc = xf32.rearrange("(k p) (t n) -> p k t n", p=128, n=NTILE)

            sync.dma_start(
                wdiff.ap().rearrange("p (k m) -> p k m", k=KC),
                wdT.rearrange("(k p) m -> p k m", p=128),
            ).then_inc(wdsem, 16)
            sync.dma_start(xb_tile(0), xbsrc[:, :, 0, :]).then_inc(xbsem, 16)
            sync.dma_start(
                wg1.ap().rearrange("p (k m) -> p k m", k=KC),
                wg1T.rearrange("(k p) m -> p k m", p=128),
            ).then_inc(wsem, 16)
            sync.dma_start(wg2p.ap(), wg2pT[:, :]).then_inc(wsem, 16)
            sync.dma_start(mpr.ap(), mpres[:, :]).then_inc(wsem, 16)
            sync.dma_start(bias_sb.ap(), biasd[:, :]).then_inc(wsem, 16)
            sync.dma_start(xb_tile(1), xbsrc[:, :, 1, :]).then_inc(xbsem, 16)
            sync.dma_start(xf_tile(0), xfsrc[:, :, 0, :]).then_inc(xfsem, 16)
            sync.dma_start(xb_tile(2), xbsrc[:, :, 2, :]).then_inc(xbsem, 16)
            sync.dma_start(xf_tile(1), xfsrc[:, :, 1, :]).then_inc(xfsem, 16)

            zdst = zoT.rearrange("(j p) (t n) -> p j t n", p=128, n=NTILE)
            for t in range(NT):
                sync.wait_ge(dve_sem, dve_i[("add", t, JC - 1)])
                sync.dma_start(zdst[:, :, t, :], zo_tile(t)).then_inc(osem, 16)
                if t + XBUF < NT:
                    sync.dma_start(xb_tile(t + XBUF),
                                   xbsrc[:, :, t + XBUF, :]).then_inc(xbsem, 16)
                if t + 2 < NT:
                    sync.dma_start(xf_tile(t + 2),
                                   xfsrc[:, :, t + 2, :]).then_inc(xfsem, 16)
            sync.wait_ge(dve_sem, dve_i[("rmax", NT - 1)])
            sync.dma_start(mlc8[:, :], mlc_sb.ap()).then_inc(osem, 16)
            sync.wait_ge(osem, 16 * (NT + 1))

        # ---- PE: all matmuls (bf16 in, fp32 accumulate) ----
        @block.tensor
        def _(tensor):
            def score(t):
                tensor.wait_ge(xbsem, 16 * (t + 1))
                if t >= 2:
                    tensor.wait_ge(act_sem, act_i[("psig", t - 2)])
                for k in range(KC):
                    mm = nc.tensor.matmul(
                        sd_sl(t), wd_sl(k), xb_sl(t, k),
                        start=(k == 0), stop=(k == KC - 1))
                mm.then_inc(pe_sem, 1)

            tensor.wait_ge(wdsem, 16)
            score(0)
            tensor.wait_ge(wsem, 64)
            for t in range(NT):
                if t + 1 < NT:
                    score(t + 1)
                tensor.wait_ge(act_sem, act_i[("psig", t)])
                for j in range(JC):
                    # gate_pre_j = sum_k Wg1[k,j]^T x[k] + Wg2p[j]^T p
                    if j >= 2:
                        tensor.wait_ge(act_sem, act_i[("gsig", t, j - 2)])
                    elif t >= 1:
                        tensor.wait_ge(act_sem, act_i[("gsig", t - 1, j + JC - 2)])
                    for k in range(KC):
                        nc.tensor.matmul(
                            gate_sl(j), wg1_sl(k, j), xb_sl(t, k),
                            start=(k == 0), stop=False)
                    mm = nc.tensor.matmul(
                        gate_sl(j), wg2p.ap()[:, j * 128:(j + 1) * 128],
                        p_sl(t), start=False, stop=True)
                    mm.then_inc(pe_sem, 1)
                    # R_j = M1[j]^T p
                    if j >= 2:
                        tensor.wait_ge(dve_sem, dve_i[("mul", t, j - 2)])
                    elif t >= 1:
                        tensor.wait_ge(dve_sem, dve_i[("mul", t - 1, j + JC - 2)])
                    mm = nc.tensor.matmul(
                        r_sl(j), mpr.ap()[:, j * 128:(j + 1) * 128],
                        p_sl(t), start=True, stop=True)
                    mm.then_inc(pe_sem, 1)

        # ---- ACT: sigmoids ----
        @block.scalar
        def _(scalar):
            for t in range(NT):
                scalar.wait_ge(pe_sem, pe_i[("score", t)])
                if t >= 2:
                    scalar.wait_ge(pe_sem, pe_i[("r", t - 2, JC - 1)])
                    scalar.wait_ge(dve_sem, dve_i[("rmax", t - 2)])
                nc.scalar.activation(
                    p_sl(t), sd_sl(t), ACTF.Sigmoid,
                    scale=SCALE).then_inc(act_sem, 1)
                for j in range(JC):
                    scalar.wait_ge(pe_sem, pe_i[("gate", t, j)])
                    if j >= 2:
                        scalar.wait_ge(dve_sem, dve_i[("mul", t, j - 2)])
                    elif t >= 1:
                        scalar.wait_ge(dve_sem, dve_i[("mul", t - 1, j + JC - 2)])
                    nc.scalar.activation(
                        g_sl(j), gate_sl(j), ACTF.Sigmoid,
                        bias=bias_sb.ap()[:, j:j + 1]).then_inc(act_sem, 1)

        # ---- DVE: max-reduce + elementwise epilogue ----
        @block.vector
        def _(vector):
            for t in range(NT):
                vector.wait_ge(act_sem, act_i[("psig", t)])
                nc.vector.tensor_reduce(
                    mlc_sb.ap()[:, t:t + 1], p_sl(t), axis=AX.X,
                    op=ALU.max).then_inc(dve_sem, 1)
                if t >= 2:
                    vector.wait_ge(osem, 16 * (t - 1))
                vector.wait_ge(xfsem, 16 * (t + 1))
                for j in range(JC):
                    vector.wait_ge(act_sem, act_i[("gsig", t, j)])
                    vector.wait_ge(pe_sem, pe_i[("r", t, j)])
                    nc.vector.tensor_mul(
                        zo_sl(t, j), g_sl(j), r_sl(j)).then_inc(dve_sem, 1)
                    nc.vector.tensor_add(
                        zo_sl(t, j), zo_sl(t, j),
                        xf_sl(t, j)).then_inc(dve_sem, 1)

    return nc


def _get_program():
    if "nc" not in _prog_cache:
        _prog_cache["nc"] = _build_program()
    return _prog_cache["nc"]


def kernel(z_fused, disease_knowledge, gate_w, gate_b):
    z_fused = np.asarray(z_fused, dtype=np.float32)
    dk = np.asarray(disease_knowledge, dtype=np.float32)
    gate_w = np.asarray(gate_w, dtype=np.float32)
    gate_b = np.asarray(gate_b, dtype=np.float32)

    m0 = dk[:, 0, :]                                    # [14, D]
    m1 = dk[:, 1, :]                                    # [14, D]
    wdT = np.ascontiguousarray((m1 - m0).T).astype(NP_BF16)     # [D, 14]
    wg1T = np.ascontiguousarray(gate_w[:, :D].T).astype(NP_BF16)  # [D, D]
    wg2pT = (m1 @ gate_w[:, D:].T).astype(NP_BF16)              # [14, D]
    mpres = m1.astype(NP_BF16)                                  # [14, D]
    biasd = np.ascontiguousarray(gate_b.reshape(JC, 128).T)     # [128, JC]

    nc = _get_program()
    in_maps = []
    for c in range(CORES):
        xs = np.ascontiguousarray(
            z_fused[c * B_LOC:(c + 1) * B_LOC].reshape(T, D).T)  # [D, T]
        in_maps.append({
            "xb16": xs.astype(NP_BF16), "xf32": xs,
            "wg1T": wg1T, "wdT": wdT, "wg2pT": wg2pT, "mpres": mpres,
            "biasd": biasd,
        })

    res = run_bass_kernel_spmd(nc, in_maps, list(range(CORES)))

    z_out = np.empty((B, S, D), dtype=np.float32)
    mlc = np.empty((B, ND), dtype=np.float32)
    for c in range(CORES):
        r = res.results[c]
        z_out[c * B_LOC:(c + 1) * B_LOC] = np.ascontiguousarray(
            r["zoT"].T).reshape(B_LOC, S, D)
        m8 = r["mlc8"]                      # [14, NT] max per 512-token tile
        per_b = m8.reshape(ND, B_LOC, NT // B_LOC).max(axis=2)  # [14, 4]
        mlc[c * B_LOC:(c + 1) * B_LOC] = per_b.T
    return z_out, mlc


# revision 10
# speedup vs baseline: 1.1839x; 1.0243x over previous
"""Trainium2 Bass kernel for DiseaseKnowledgeModule.

Math (per token z in R^d, d=1024, 14 diseases x 2 states):
  score = z @ flat_memory.T / sqrt(d)                    [28]
  p     = softmax over states [:, 1] == sigmoid(s1 - s0) [14]
  mlc   = max over patches of p                          [B, 14]
  R     = p @ M_present                                  [d]
  gate  = sigmoid([z, R] @ gate_w.T + gate_b)            [d]
  z_out = z + gate * R

Device formulation (feature-major / transposed activations):
  sd    = Wdiff contraction: (m1 - m0) . z               one matmul, K=d
  p     = sigmoid(sd / sqrt(d))
  gate^T= sigmoid(Wg1 @ z^T + (M1 @ Wg2^T)^T-fold @ p^T + b)   K=d + K=14
  R^T   = M1-contraction: K=14
  zout^T= z^T + gate^T * R^T

Sharding: data-parallel over batch B=32 across 8 cores (4 batches/core).
Matmuls in bf16 (fp32 PSUM accumulate); residual add in fp32.
"""
import sys
import numpy as np

sys.path.insert(0, "/opt/trn_rl_repo")

import ml_dtypes
import concourse.bass as bass
import concourse.mybir as mybir
from concourse.bass_utils import run_bass_kernel_spmd

F32 = mybir.dt.float32
BF16 = mybir.dt.bfloat16
AX = mybir.AxisListType
ALU = mybir.AluOpType
ACTF = mybir.ActivationFunctionType
NP_BF16 = ml_dtypes.bfloat16

B, S, D = 32, 1024, 1024
ND, NS = 14, 2
CORES = 8
B_LOC = B // CORES            # 4 batches per core
T = B_LOC * S                 # 4096 tokens per core
NTILE = 512                   # tokens per T-tile (one PSUM bank of fp32)
NT = T // NTILE               # 8 T-tiles
KC = D // 128                 # 8 contraction chunks of 128
JC = D // 128                 # 8 output d-tiles of 128
XBUF = 3                      # bf16 x tile buffers
SCALE = 1.0 / float(np.sqrt(np.float32(D)))

_prog_cache = {}


def _op_numbering():
    """Per-engine op indices (1-based cumulative semaphore values)."""
    pe, act, dve = {}, {}, {}
    n = 0
    n += 1; pe[("score", 0)] = n
    for t in range(NT):
        if t + 1 < NT:
            n += 1; pe[("score", t + 1)] = n
        for j in range(JC):
            n += 1; pe[("gate", t, j)] = n
            n += 1; pe[("r", t, j)] = n
    n = 0
    for t in range(NT):
        n += 1; act[("psig", t)] = n
        if t == NT - 1:
            n += 1; act[("mlcsig",)] = n
        for j in range(JC):
            n += 1; act[("gsig", t, j)] = n
    n = 0
    for t in range(NT):
        n += 1; dve[("rmax", t)] = n
        for j in range(JC):
            n += 1; dve[("mul", t, j)] = n
            n += 1; dve[("add", t, j)] = n
    return pe, act, dve


def _build_program():
    from concourse.bass import compact_to_ranges

    pe_i, act_i, dve_i = _op_numbering()
    nc = bass.Bass()

    # A previous NEFF on this core (e.g. an XLA graph) may leave kernel-range
    # semaphores nonzero; our waits use absolute values and assume they start
    # at 0. Mirror the target_bir_lowering=True prologue: clear the whole
    # kernel sem range on gpsimd, then hold every engine at an NRT-level
    # barrier (which does not depend on bass sems) until the clear lands.
    for sem_range in compact_to_ranges(
            [s for s in nc._kernel_sem_range if s not in nc.barrier_sems]):
        nc.gpsimd.dma_reset(sem_range)
        nc.gpsimd.sem_clear(sem_range)
    nc._nrt_pseudo_barrier()

    xb16 = nc.dram_tensor("xb16", [D, T], BF16, kind="ExternalInput")
    xf32 = nc.dram_tensor("xf32", [D, T], F32, kind="ExternalInput")
    wg1T = nc.dram_tensor("wg1T", [D, D], BF16, kind="ExternalInput")
    wdT = nc.dram_tensor("wdT", [D, ND], BF16, kind="ExternalInput")
    wg2pT = nc.dram_tensor("wg2pT", [ND, D], BF16, kind="ExternalInput")
    mpres = nc.dram_tensor("mpres", [ND, D], BF16, kind="ExternalInput")
    biasd = nc.dram_tensor("biasd", [128, JC], F32, kind="ExternalInput")
    zoT = nc.dram_tensor("zoT", [D, T], F32, kind="ExternalOutput")
    mlc8 = nc.dram_tensor("mlc8", [ND, NT], F32, kind="ExternalOutput")

    from contextlib import ExitStack
    with ExitStack() as ctx:
        xb = ctx.enter_context(nc.sbuf_tensor("xb", [128, XBUF * KC * NTILE], BF16))
        xf = ctx.enter_context(nc.sbuf_tensor("xf", [128, 2 * KC * NTILE], F32))
        wg1 = ctx.enter_context(nc.sbuf_tensor("wg1", [128, KC * D], BF16))
        wdiff = ctx.enter_context(nc.sbuf_tensor("wdiff", [128, KC * ND], BF16))
        wg2p = ctx.enter_context(nc.sbuf_tensor("wg2p", [ND, D], BF16))
        mpr = ctx.enter_context(nc.sbuf_tensor("mpr", [ND, D], BF16))
        bias_sb = ctx.enter_context(nc.sbuf_tensor("bias_sb", [128, JC], F32))
        p_sb = ctx.enter_context(nc.sbuf_tensor("p_sb", [ND, 2 * NTILE], BF16))
        g_sb = ctx.enter_context(nc.sbuf_tensor("g_sb", [128, 2 * NTILE], F32))
        zo = ctx.enter_context(nc.sbuf_tensor("zo", [128, 2 * JC * NTILE], F32))
        sdm_sb = ctx.enter_context(nc.sbuf_tensor("sdm_sb", [ND, NT], F32))
        mlc_sb = ctx.enter_context(nc.sbuf_tensor("mlc_sb", [ND, NT], F32))
        sd_ps = ctx.enter_context(nc.psum_tensor("sd_ps", [ND, 2 * NTILE], F32))
        gate_ps = ctx.enter_context(nc.psum_tensor("gate_ps", [128, 2 * NTILE], F32))
        r_ps = ctx.enter_context(nc.psum_tensor("r_ps", [128, 2 * NTILE], F32))
        wdsem = ctx.enter_context(nc.semaphore("wdsem"))
        wsem = ctx.enter_context(nc.semaphore("wsem"))
        xbsem = ctx.enter_context(nc.semaphore("xbsem"))
        xfsem = ctx.enter_context(nc.semaphore("xfsem"))
        osem = ctx.enter_context(nc.semaphore("osem"))
        pe_sem = ctx.enter_context(nc.semaphore("pe_sem"))
        act_sem = ctx.enter_context(nc.semaphore("act_sem"))
        dve_sem = ctx.enter_context(nc.semaphore("dve_sem"))
        block = ctx.enter_context(nc.Block(no_gpsimd_drain=True))

        # ---- slice helpers ----
        def xb_sl(t, k):        # bf16 rhs [128, NTILE] for k-chunk of tile t
            o = (t % XBUF) * KC * NTILE + k * NTILE
            return xb.ap()[:, o:o + NTILE]

        def xb_tile(t):
            o = (t % XBUF) * KC * NTILE
            return xb.ap()[:, o:o + KC * NTILE].rearrange(
                "p (k n) -> p k n", k=KC)

        def xf_sl(t, j):        # f32 residual [128, NTILE] for d-tile j
            o = (t % 2) * KC * NTILE + j * NTILE
            return xf.ap()[:, o:o + NTILE]

        def xf_tile(t):
            o = (t % 2) * KC * NTILE
            return xf.ap()[:, o:o + KC * NTILE].rearrange(
                "p (k n) -> p k n", k=KC)

        def wg1_sl(k, j):
            o = k * D + j * 128
            return wg1.ap()[:, o:o + 128]

        def wd_sl(k):
            o = k * ND
            return wdiff.ap()[:, o:o + ND]

        def p_sl(t):
            o = (t % 2) * NTILE
            return p_sb.ap()[:, o:o + NTILE]

        def g_sl(j):
            o = (j % 2) * NTILE
            return g_sb.ap()[:, o:o + NTILE]

        def zo_sl(t, j):
            o = (t % 2) * JC * NTILE + j * NTILE
            return zo.ap()[:, o:o + NTILE]

        def zo_tile(t):
            o = (t % 2) * JC * NTILE
            return zo.ap()[:, o:o + JC * NTILE].rearrange(
                "p (j n) -> p j n", j=JC)

        def sd_sl(t):
            o = (t % 2) * NTILE
            return sd_ps.ap()[:, o:o + NTILE]

        def gate_sl(j):
            o = (j % 2) * NTILE
            return gate_ps.ap()[:, o:o + NTILE]

        def r_sl(j):
            o = (j % 2) * NTILE
            return r_ps.ap()[:, o:o + NTILE]

        # ---- SP: all DMA ----
        @block.sync
        def _(sync):
            xbsrc = xb16.rearrange("(k p) (t n) -> p k t n", p=128, n=NTILE)
            xfsrc = xf32.rearrange("(k p) (t n) -> p k t n", p=128, n=NTILE)

            sync.dma_start(xb_tile(0), xbsrc[:, :, 0, :]).then_inc(xbsem, 16)
            sync.dma_start(
                wdiff.ap().rearrange("p (k m) -> p k m", k=KC),
                wdT.rearrange("(k p) m -> p k m", p=128),
            ).then_inc(wdsem, 16)
            sync.dma_start(bias_sb.ap(), biasd[:, :]).then_inc(wdsem, 16)
            sync.dma_start(
                wg1.ap().rearrange("p (k m) -> p k m", k=KC),
                wg1T.rearrange("(k p) m -> p k m", p=128),
            ).then_inc(wsem, 16)
            sync.dma_start(wg2p.ap(), wg2pT[:, :]).then_inc(wsem, 16)
            sync.dma_start(mpr.ap(), mpres[:, :]).then_inc(wsem, 16)
            sync.dma_start(xb_tile(1), xbsrc[:, :, 1, :]).then_inc(xbsem, 16)
            sync.dma_start(xf_tile(0), xfsrc[:, :, 0, :]).then_inc(xfsem, 16)
            sync.dma_start(xb_tile(2), xbsrc[:, :, 2, :]).then_inc(xbsem, 16)
            sync.dma_start(xf_tile(1), xfsrc[:, :, 1, :]).then_inc(xfsem, 16)

            zdst = zoT.rearrange("(j p) (t n) -> p j t n", p=128, n=NTILE)
            for t in range(NT - 1):
                sync.wait_ge(dve_sem, dve_i[("add", t, JC - 1)])
                sync.dma_start(zdst[:, :, t, :], zo_tile(t)).then_inc(osem, 16)
                if t + XBUF < NT:
                    sync.dma_start(xb_tile(t + XBUF),
                                   xbsrc[:, :, t + XBUF, :]).then_inc(xbsem, 16)
                if t + 2 < NT:
                    sync.dma_start(xf_tile(t + 2),
                                   xfsrc[:, :, t + 2, :]).then_inc(xfsem, 16)
            # last tile: stream each d-slice out as soon as its add lands,
            # and let the (tiny) mlc transfer fly as early as possible
            tl = NT - 1
            sync.wait_ge(act_sem, act_i[("mlcsig",)])
            sync.dma_start(mlc8[:, :], mlc_sb.ap()).then_inc(osem, 16)
            for j in range(JC):
                sync.wait_ge(dve_sem, dve_i[("add", tl, j)])
                sync.dma_start(zdst[:, j, tl, :],
                               zo_sl(tl, j)).then_inc(osem, 16)
            sync.wait_ge(osem, 16 * (NT + JC))

        # ---- PE: all matmuls (bf16 in, fp32 accumulate) ----
        @block.tensor
        def _(tensor):
            def score(t):
                tensor.wait_ge(xbsem, 16 * (t + 1))
                if t >= 2:
                    tensor.wait_ge(act_sem, act_i[("psig", t - 2)])
                    tensor.wait_ge(dve_sem, dve_i[("rmax", t - 2)])
                for k in range(KC):
                    mm = nc.tensor.matmul(
                        sd_sl(t), wd_sl(k), xb_sl(t, k),
                        start=(k == 0), stop=(k == KC - 1))
                mm.then_inc(pe_sem, 1)

            tensor.wait_ge(wdsem, 16)
            score(0)
            tensor.wait_ge(wsem, 48)
            for t in range(NT):
                if t + 1 < NT:
                    score(t + 1)
                tensor.wait_ge(act_sem, act_i[("psig", t)])
                for j in range(JC):
                    # gate_pre_j = sum_k Wg1[k,j]^T x[k] + Wg2p[j]^T p
                    if j >= 2:
                        tensor.wait_ge(act_sem, act_i[("gsig", t, j - 2)])
                    elif t >= 1:
                        tensor.wait_ge(act_sem, act_i[("gsig", t - 1, j + JC - 2)])
                    for k in range(KC):
                        nc.tensor.matmul(
                            gate_sl(j), wg1_sl(k, j), xb_sl(t, k),
                            start=(k == 0), stop=False)
                    mm = nc.tensor.matmul(
                        gate_sl(j), wg2p.ap()[:, j * 128:(j + 1) * 128],
                        p_sl(t), start=False, stop=True)
                    mm.then_inc(pe_sem, 1)
                    # R_j = M1[j]^T p
                    if j >= 2:
                        tensor.wait_ge(dve_sem, dve_i[("mul", t, j - 2)])
                    elif t >= 1:
                        tensor.wait_ge(dve_sem, dve_i[("mul", t - 1, j + JC - 2)])
                    mm = nc.tensor.matmul(
                        r_sl(j), mpr.ap()[:, j * 128:(j + 1) * 128],
                        p_sl(t), start=True, stop=True)
                    mm.then_inc(pe_sem, 1)

        # ---- ACT: sigmoids ----
        @block.scalar
        def _(scalar):
            scalar.wait_ge(wdsem, 32)
            for t in range(NT):
                scalar.wait_ge(pe_sem, pe_i[("score", t)])
                if t >= 2:
                    scalar.wait_ge(pe_sem, pe_i[("r", t - 2, JC - 1)])
                nc.scalar.activation(
                    p_sl(t), sd_sl(t), ACTF.Sigmoid,
                    scale=SCALE).then_inc(act_sem, 1)
                if t == NT - 1:
                    scalar.wait_ge(dve_sem, dve_i[("rmax", NT - 1)])
                    nc.scalar.activation(
                        mlc_sb.ap(), sdm_sb.ap(), ACTF.Sigmoid,
                        scale=SCALE).then_inc(act_sem, 1)
                for j in range(JC):
                    scalar.wait_ge(pe_sem, pe_i[("gate", t, j)])
                    if j >= 2:
                        scalar.wait_ge(dve_sem, dve_i[("mul", t, j - 2)])
                    elif t >= 1:
                        scalar.wait_ge(dve_sem, dve_i[("mul", t - 1, j + JC - 2)])
                    nc.scalar.activation(
                        g_sl(j), gate_sl(j), ACTF.Sigmoid,
                        bias=bias_sb.ap()[:, j:j + 1]).then_inc(act_sem, 1)


        # ---- DVE: max-reduce + elementwise epilogue ----
        @block.vector
        def _(vector):
            for t in range(NT):
                vector.wait_ge(pe_sem, pe_i[("score", t)])
                nc.vector.tensor_reduce(
                    sdm_sb.ap()[:, t:t + 1], sd_sl(t), axis=AX.X,
                    op=ALU.max).then_inc(dve_sem, 1)
                if t >= 2:
                    vector.wait_ge(osem, 16 * (t - 1))
                vector.wait_ge(xfsem, 16 * (t + 1))
                for j in range(JC):
                    vector.wait_ge(act_sem, act_i[("gsig", t, j)])
                    vector.wait_ge(pe_sem, pe_i[("r", t, j)])
                    nc.vector.tensor_mul(
                        zo_sl(t, j), g_sl(j), r_sl(j)).then_inc(dve_sem, 1)
                    nc.vector.tensor_add(
                        zo_sl(t, j), zo_sl(t, j),
                        xf_sl(t, j)).then_inc(dve_sem, 1)

    return nc


def _get_program():
    if "nc" not in _prog_cache:
        _prog_cache["nc"] = _build_program()
    return _prog_cache["nc"]


def kernel(z_fused, disease_knowledge, gate_w, gate_b):
    z_fused = np.asarray(z_fused, dtype=np.float32)
    dk = np.asarray(disease_knowledge, dtype=np.float32)
    gate_w = np.asarray(gate_w, dtype=np.float32)
    gate_b = np.asarray(gate_b, dtype=np.float32)

    m0 = dk[:, 0, :]                                    # [14, D]
    m1 = dk[:, 1, :]                                    # [14, D]
    wdT = np.ascontiguousarray((m1 - m0).T).astype(NP_BF16)     # [D, 14]
    wg1T = np.ascontiguousarray(gate_w[:, :D].T).astype(NP_BF16)  # [D, D]
    wg2pT = (m1 @ gate_w[:, D:].T).astype(NP_BF16)              # [14, D]
    mpres = m1.astype(NP_BF16)                                  # [14, D]
    biasd = np.ascontiguousarray(gate_b.reshape(JC, 128).T)     # [128, JC]

    nc = _get_program()
    in_maps = []
    for c in range(CORES):
        xs = np.ascontiguousarray(
            z_fused[c * B_LOC:(c + 1) * B_LOC].reshape(T, D).T)  # [D, T]
        in_maps.append({
            "xb16": xs.astype(NP_BF16), "xf32": xs,
            "wg1T": wg1T, "wdT": wdT, "wg2pT": wg2pT, "mpres": mpres,
            "biasd": biasd,
        })

    res = run_bass_kernel_spmd(nc, in_maps, list(range(CORES)))

    z_out = np.empty((B, S, D), dtype=np.float32)
    mlc = np.empty((B, ND), dtype=np.float32)
    for c in range(CORES):
        r = res.results[c]
        z_out[c * B_LOC:(c + 1) * B_LOC] = np.ascontiguousarray(
            r["zoT"].T).reshape(B_LOC, S, D)
        m8 = r["mlc8"]                      # [14, NT] max per 512-token tile
        per_b = m8.reshape(ND, B_LOC, NT // B_LOC).max(axis=2)  # [14, 4]
        mlc[c * B_LOC:(c + 1) * B_LOC] = per_b.T
    return z_out, mlc
